# revision 16
# baseline (speedup 1.0000x reference)
"""Expert-parallel MoE kernel for Trainium2 (8 NeuronCores).

Strategy (per the expert-parallel sharding hint):
  - Host computes router logits / top-2 routing in numpy (0.02% of FLOPs);
    this decides the token->core all-to-all. Each expert's tokens are
    gathered into a zero-padded, transposed shard x_eT [D, CAP].
  - Core e holds w1[e], w2[e] resident in SBUF and computes
        y = cw * (silu(x @ w1[e]) @ w2[e])
    for its tokens with float32r matmuls accumulating in fp32 PSUM.
    Layer 1 computes H^T = w1^T x^T directly (tokens on the free dim), so
    no transposes are needed anywhere: lhsT/rhs of both matmuls are the
    natural storage layouts.
  - Host scatter-adds the per-expert outputs back into [T, D].

DMA plan: x chunks + combine weights ride the Scalar-engine HWDGE queue
(host pre-tiles x so each transfer is contiguous per partition); w1
(per-128-row tiles) then w2 stream FIFO on the Sync-engine HWDGE queue, so
early weight tiles land early and pace chunk-0 compute; y tiles drain on
the GpSimd queue. Chunk 0's layer-1 matmuls are emitted d-major over
4-f-group blocks so the PE makes progress as each w1 row-tile arrives.
"""

import numpy as np

B, S, D, F, E, TOP_K = 2, 2048, 1024, 2048, 8, 2
T = B * S
P = 128

_KERNEL_CACHE: dict = {}


def _build_bass(cap: int, chunk: int):
    """Build the per-core Bass program for capacity `cap` (multiple of `chunk`,
    itself a multiple of 128)."""
    from contextlib import ExitStack

    import concourse.bacc as bacc
    import concourse.mybir as mybir
    import concourse.tile as tile
    f32 = mybir.dt.float32
    f32r = mybir.dt.float32r

    KD = D // P            # 8  k-tiles (layer 1 contraction over D)
    KF = F // P            # 16 k-tiles (layer 2 contraction over F)
    n_chunks = cap // chunk
    MT = chunk // P        # m-tiles (of 128 tokens) per chunk
    FB = 4                 # chunk-0 trickle f-group block

    nc = bacc.Bacc("TRN2", target_bir_lowering=False)
    xT_d = nc.dram_tensor("x_t", [P, cap // chunk, D // P, chunk], f32r, kind="ExternalInput")
    w1_d = nc.dram_tensor("w1e", [D, F], f32r, kind="ExternalInput")
    w2_d = nc.dram_tensor("w2e", [F, D], f32r, kind="ExternalInput")
    cw_d = nc.dram_tensor("cw_t", [P, cap // P], f32, kind="ExternalInput")
    y_d = nc.dram_tensor("y", [cap, D], f32, kind="ExternalOutput")

    with ExitStack() as ctx:
        tc = ctx.enter_context(tile.TileContext(nc))
        wpool = ctx.enter_context(tc.tile_pool(name="weights", bufs=1))
        xpool = ctx.enter_context(tc.tile_pool(name="x", bufs=1))
        htpool = ctx.enter_context(tc.tile_pool(name="ht", bufs=2))
        ypool = ctx.enter_context(tc.tile_pool(name="y", bufs=2))
        l1ps = ctx.enter_context(tc.tile_pool(name="l1ps", bufs=4, space="PSUM"))
        l2ps = ctx.enter_context(tc.tile_pool(name="l2ps", bufs=4, space="PSUM"))

        # PE warm-up: ~5us of scratch matmuls while the first DMAs are in
        # flight, so the HAM clock gate is at 2.4GHz when real work starts.
        # Results land in an l2ps scratch bank (idle until ~45us) and are
        # discarded; the first real accumulation group re-inits the bank
        # (start=True clears has_written).
        warm_sb = wpool.tile([P, 512], f32, tag="warm")
        nc.vector.memset(warm_sb[:], 1.0)
        warm_ps = l2ps.tile([P, 512], f32, tag="l2", name="warm_ps")
        for wi in range(6):
            nc.tensor.matmul(
                warm_ps[:], warm_sb[:, 0:P], warm_sb[:, 0:512],
                start=(wi == 0), stop=(wi == 5),
            )

        # x chunks on the Scalar HWDGE queue (contiguous per partition thanks
        # to host pre-tiling) so layer 1 starts within a few us
        x_tiles = []
        for ci in range(n_chunks):
            x_sb = xpool.tile([P, KD, chunk], f32r, tag="x", name=f"x_{ci}")
            nc.scalar.dma_start(x_sb[:], xT_d[:, ci, :, :])
            x_tiles.append(x_sb)
        cw_sb = wpool.tile([P, cap // P], f32, tag="cw")
        nc.scalar.dma_start(cw_sb[:], cw_d[:, :])

        # weights stream w1_0..w1_7, w2_0..w2_15 FIFO on the Sync queue
        w1_t = [wpool.tile([P, F], f32r, tag=f"w1_{d}", name=f"w1_{d}")
                for d in range(KD)]
        w2_t = [wpool.tile([P, D], f32r, tag=f"w2_{k}", name=f"w2_{k}")
                for k in range(KF)]
        for d in range(KD):
            nc.sync.dma_start(w1_t[d][:], w1_d[d * P:(d + 1) * P, :])
        for k in range(KF):
            nc.sync.dma_start(w2_t[k][:], w2_d[k * P:(k + 1) * P, :])

        for ci in range(n_chunks):
            x_sb = x_tiles[ci]
            ht_sb = htpool.tile([P, KF, chunk], f32r, tag="ht")

            if ci == 0:
                # d-major over blocks of FB f-groups: PE progresses as each
                # w1 row-tile lands instead of waiting for all of w1
                for fb in range(0, KF, FB):
                    fs = list(range(fb, fb + FB))
                    pss = [l1ps.tile([P, chunk], f32, tag="l1", name=f"ps{f}")
                           for f in fs]
                    for d in range(KD):
                        for j, f in enumerate(fs):
                            nc.tensor.matmul(
                                pss[j][:], w1_t[d][:, f * P:(f + 1) * P],
                                x_sb[:, d, :],
                                start=(d == 0), stop=(d == KD - 1),
                            )
                    for j, f in enumerate(fs):
                        nc.scalar.activation(
                            ht_sb[:, f, :], pss[j][:],
                            mybir.ActivationFunctionType.Silu,
                        )
            else:
                for f in range(KF):
                    ps = l1ps.tile([P, chunk], f32, tag="l1")
                    for d in range(KD):
                        nc.tensor.matmul(
                            ps[:], w1_t[d][:, f * P:(f + 1) * P], x_sb[:, d, :],
                            start=(d == 0), stop=(d == KD - 1),
                        )
                    nc.scalar.activation(
                        ht_sb[:, f, :], ps[:], mybir.ActivationFunctionType.Silu
                    )

            for m in range(MT):
                gm = ci * MT + m
                ps0 = l2ps.tile([P, 512], f32, tag="l2")
                ps1 = l2ps.tile([P, 512], f32, tag="l2")
                for k in range(KF):
                    nc.tensor.matmul(
                        ps0[:], ht_sb[:, k, m * P:(m + 1) * P], w2_t[k][:, 0:512],
                        start=(k == 0), stop=(k == KF - 1),
                    )
                    nc.tensor.matmul(
                        ps1[:], ht_sb[:, k, m * P:(m + 1) * P], w2_t[k][:, 512:1024],
                        start=(k == 0), stop=(k == KF - 1),
                    )
                y_sb0 = ypool.tile([P, 512], f32, tag="y", name="y0")
                y_sb1 = ypool.tile([P, 512], f32, tag="y", name="y1")
                nc.vector.tensor_scalar_mul(y_sb0[:], ps0[:], cw_sb[:, gm:gm + 1])
                nc.vector.tensor_scalar_mul(y_sb1[:], ps1[:], cw_sb[:, gm:gm + 1])
                nc.gpsimd.dma_start(y_d[gm * P:(gm + 1) * P, 0:512], y_sb0[:])
                nc.gpsimd.dma_start(y_d[gm * P:(gm + 1) * P, 512:1024], y_sb1[:])

    nc.compile()
    return nc


def _routing(x: np.ndarray, gate_w: np.ndarray):
    """Replicate the reference's fp32 routing on host."""
    logits = x @ gate_w.T                              # [T, E] fp32
    lm = logits.max(axis=-1, keepdims=True)
    p = np.exp(logits - lm)
    probs = p / p.sum(axis=-1, keepdims=True)          # softmax fp32
    order = np.argsort(-probs, axis=-1, kind="stable")
    sel = order[:, :TOP_K]                             # [T, K]
    topw = np.take_along_axis(probs, sel, axis=-1)
    topw = topw / topw.sum(axis=-1, keepdims=True)
    return logits, sel, topw.astype(np.float32)


def kernel(hidden_states, gate_w, w1, w2):
    from concourse.bass_utils import run_bass_kernel_spmd

    hidden_states = np.asarray(hidden_states, dtype=np.float32)
    gate_w = np.asarray(gate_w, dtype=np.float32)
    w1 = np.asarray(w1, dtype=np.float32)
    w2 = np.asarray(w2, dtype=np.float32)

    x = hidden_states.reshape(T, D)
    logits, sel, topw = _routing(x, gate_w)

    # Per-expert token lists
    idxs, cws = [], []
    for e in range(E):
        hit = (sel == e)                                # [T, K]
        tok = np.nonzero(hit.any(axis=1))[0]
        k_of = np.argmax(hit[tok], axis=1)
        idxs.append(tok)
        cws.append(topw[tok, k_of])
    max_n = max(len(i) for i in idxs)

    chunk = 384
    cap = -(-max_n // chunk) * chunk                    # round up to chunk multiple

    key = (cap, chunk)
    if key not in _KERNEL_CACHE:
        _KERNEL_CACHE[key] = _build_bass(cap, chunk)
    nc = _KERNEL_CACHE[key]

    in_maps = []
    for e in range(E):
        n_e = len(idxs[e])
        xT_pad = np.zeros((D, cap), np.float32)
        xT_pad[:, :n_e] = x[idxs[e]].T
        # pretile to [P, n_chunks, KD, chunk]: row d = kt*P + p
        xt4 = np.ascontiguousarray(
            xT_pad.reshape(D // P, P, cap // chunk, chunk).transpose(1, 2, 0, 3))
        cw_pad = np.zeros(cap, np.float32)
        cw_pad[:n_e] = cws[e]
        in_maps.append({
            "x_t": xt4,
            "w1e": np.ascontiguousarray(w1[e]),
            "w2e": np.ascontiguousarray(w2[e]),
            "cw_t": np.ascontiguousarray(cw_pad.reshape(cap // P, P).T),
        })

    res = run_bass_kernel_spmd(nc, in_maps, core_ids=list(range(E)))
    globals()["LAST_RESULTS"] = res

    out = np.zeros((T, D), np.float32)
    for e in range(E):
        n_e = len(idxs[e])
        out[idxs[e]] += res.results[e]["y"][:n_e]

    return out.reshape(B, S, D), logits


# revision 18
# speedup vs baseline: 1.0341x; 1.0341x over previous
"""Expert-parallel MoE kernel for Trainium2 (8 NeuronCores).

Strategy (per the expert-parallel sharding hint):
  - Host computes router logits / top-2 routing in numpy (0.02% of FLOPs);
    this decides the token->core all-to-all. Each expert's tokens are
    gathered into a zero-padded, transposed shard x_eT [D, CAP].
  - Core e holds w1[e], w2[e] resident in SBUF and computes
        y = cw * (silu(x @ w1[e]) @ w2[e])
    for its tokens with float32r matmuls accumulating in fp32 PSUM.
    Layer 1 computes H^T = w1^T x^T directly (tokens on the free dim), so
    no transposes are needed anywhere: lhsT/rhs of both matmuls are the
    natural storage layouts.
  - Host scatter-adds the per-expert outputs back into [T, D].

DMA plan: x chunks + combine weights ride the Scalar-engine HWDGE queue
(host pre-tiles x so each transfer is contiguous per partition); w1
(per-128-row tiles) then w2 stream FIFO on the Sync-engine HWDGE queue, so
early weight tiles land early and pace chunk-0 compute; y tiles drain on
the GpSimd queue. Chunk 0's layer-1 matmuls are emitted d-major over
4-f-group blocks so the PE makes progress as each w1 row-tile arrives.
"""

import numpy as np

B, S, D, F, E, TOP_K = 2, 2048, 1024, 2048, 8, 2
T = B * S
P = 128

_KERNEL_CACHE: dict = {}


def _build_bass(cap: int, chunk: int):
    """Build the per-core Bass program for capacity `cap` (multiple of `chunk`,
    itself a multiple of 128)."""
    from contextlib import ExitStack

    import concourse.bacc as bacc
    import concourse.mybir as mybir
    import concourse.tile as tile
    f32 = mybir.dt.float32
    f32r = mybir.dt.float32r

    KD = D // P            # 8  k-tiles (layer 1 contraction over D)
    KF = F // P            # 16 k-tiles (layer 2 contraction over F)
    n_chunks = cap // chunk
    MT = chunk // P        # m-tiles (of 128 tokens) per chunk
    FB = 4                 # chunk-0 trickle f-group block

    nc = bacc.Bacc("TRN2", target_bir_lowering=False)
    xT_d = nc.dram_tensor("x_t", [P, cap // chunk, D // P, chunk], f32r, kind="ExternalInput")
    w1_d = nc.dram_tensor("w1e", [D, F], f32r, kind="ExternalInput")
    w2_d = nc.dram_tensor("w2e", [F, D], f32r, kind="ExternalInput")
    cw_d = nc.dram_tensor("cw_t", [P, cap // P], f32, kind="ExternalInput")
    y_d = nc.dram_tensor("y", [cap, D], f32, kind="ExternalOutput")

    with ExitStack() as ctx:
        tc = ctx.enter_context(tile.TileContext(nc))
        wpool = ctx.enter_context(tc.tile_pool(name="weights", bufs=1))
        xpool = ctx.enter_context(tc.tile_pool(name="x", bufs=1))
        htpool = ctx.enter_context(tc.tile_pool(name="ht", bufs=2))
        ypool = ctx.enter_context(tc.tile_pool(name="y", bufs=2))
        l1ps = ctx.enter_context(tc.tile_pool(name="l1ps", bufs=4, space="PSUM"))
        l2ps = ctx.enter_context(tc.tile_pool(name="l2ps", bufs=4, space="PSUM"))

        # x chunks on the Scalar HWDGE queue (contiguous per partition thanks
        # to host pre-tiling) so layer 1 starts within a few us
        x_tiles = []
        for ci in range(n_chunks):
            x_sb = xpool.tile([P, KD, chunk], f32r, tag="x", name=f"x_{ci}")
            nc.scalar.dma_start(x_sb[:], xT_d[:, ci, :, :])
            x_tiles.append(x_sb)
        cw_sb = wpool.tile([P, cap // P], f32, tag="cw")
        nc.scalar.dma_start(cw_sb[:], cw_d[:, :])

        # weights stream w1_0..w1_7, w2_0..w2_15 FIFO on the Sync queue
        w1_t = [wpool.tile([P, F], f32r, tag=f"w1_{d}", name=f"w1_{d}")
                for d in range(KD)]
        w2_t = [wpool.tile([P, D], f32r, tag=f"w2_{k}", name=f"w2_{k}")
                for k in range(KF)]
        for d in range(KD):
            nc.sync.dma_start(w1_t[d][:], w1_d[d * P:(d + 1) * P, :])
        for k in range(KF):
            nc.sync.dma_start(w2_t[k][:], w2_d[k * P:(k + 1) * P, :])

        for ci in range(n_chunks):
            x_sb = x_tiles[ci]
            ht_sb = htpool.tile([P, KF, chunk], f32r, tag="ht")

            if ci == 0:
                # d-major over blocks of FB f-groups: PE progresses as each
                # w1 row-tile lands instead of waiting for all of w1
                for fb in range(0, KF, FB):
                    fs = list(range(fb, fb + FB))
                    pss = [l1ps.tile([P, chunk], f32, tag="l1", name=f"ps{f}")
                           for f in fs]
                    for d in range(KD):
                        for j, f in enumerate(fs):
                            nc.tensor.matmul(
                                pss[j][:], w1_t[d][:, f * P:(f + 1) * P],
                                x_sb[:, d, :],
                                start=(d == 0), stop=(d == KD - 1),
                            )
                    for j, f in enumerate(fs):
                        nc.scalar.activation(
                            ht_sb[:, f, :], pss[j][:],
                            mybir.ActivationFunctionType.Silu,
                        )
            else:
                for f in range(KF):
                    ps = l1ps.tile([P, chunk], f32, tag="l1")
                    for d in range(KD):
                        nc.tensor.matmul(
                            ps[:], w1_t[d][:, f * P:(f + 1) * P], x_sb[:, d, :],
                            start=(d == 0), stop=(d == KD - 1),
                        )
                    nc.scalar.activation(
                        ht_sb[:, f, :], ps[:], mybir.ActivationFunctionType.Silu
                    )

            for m in range(MT):
                gm = ci * MT + m
                ps0 = l2ps.tile([P, 512], f32, tag="l2")
                ps1 = l2ps.tile([P, 512], f32, tag="l2")
                for k in range(KF):
                    nc.tensor.matmul(
                        ps0[:], ht_sb[:, k, m * P:(m + 1) * P], w2_t[k][:, 0:512],
                        start=(k == 0), stop=(k == KF - 1),
                    )
                    nc.tensor.matmul(
                        ps1[:], ht_sb[:, k, m * P:(m + 1) * P], w2_t[k][:, 512:1024],
                        start=(k == 0), stop=(k == KF - 1),
                    )
                y_sb0 = ypool.tile([P, 512], f32, tag="y", name="y0")
                y_sb1 = ypool.tile([P, 512], f32, tag="y", name="y1")
                nc.vector.tensor_scalar_mul(y_sb0[:], ps0[:], cw_sb[:, gm:gm + 1])
                nc.vector.tensor_scalar_mul(y_sb1[:], ps1[:], cw_sb[:, gm:gm + 1])
                # final m-tile drains via HWDGE (lower completion latency
                # right before the end-of-kernel barrier)
                yq = nc.scalar if gm == cap // P - 1 else nc.gpsimd
                yq.dma_start(y_d[gm * P:(gm + 1) * P, 0:512], y_sb0[:])
                yq.dma_start(y_d[gm * P:(gm + 1) * P, 512:1024], y_sb1[:])

    nc.compile()
    return nc


def _routing(x: np.ndarray, gate_w: np.ndarray):
    """Replicate the reference's fp32 routing on host."""
    logits = x @ gate_w.T                              # [T, E] fp32
    lm = logits.max(axis=-1, keepdims=True)
    p = np.exp(logits - lm)
    probs = p / p.sum(axis=-1, keepdims=True)          # softmax fp32
    order = np.argsort(-probs, axis=-1, kind="stable")
    sel = order[:, :TOP_K]                             # [T, K]
    topw = np.take_along_axis(probs, sel, axis=-1)
    topw = topw / topw.sum(axis=-1, keepdims=True)
    return logits, sel, topw.astype(np.float32)


def kernel(hidden_states, gate_w, w1, w2):
    from concourse.bass_utils import run_bass_kernel_spmd

    hidden_states = np.asarray(hidden_states, dtype=np.float32)
    gate_w = np.asarray(gate_w, dtype=np.float32)
    w1 = np.asarray(w1, dtype=np.float32)
    w2 = np.asarray(w2, dtype=np.float32)

    x = hidden_states.reshape(T, D)
    logits, sel, topw = _routing(x, gate_w)

    # Per-expert token lists
    idxs, cws = [], []
    for e in range(E):
        hit = (sel == e)                                # [T, K]
        tok = np.nonzero(hit.any(axis=1))[0]
        k_of = np.argmax(hit[tok], axis=1)
        idxs.append(tok)
        cws.append(topw[tok, k_of])
    max_n = max(len(i) for i in idxs)

    chunk = 384
    cap = -(-max_n // chunk) * chunk                    # round up to chunk multiple

    key = (cap, chunk)
    if key not in _KERNEL_CACHE:
        _KERNEL_CACHE[key] = _build_bass(cap, chunk)
    nc = _KERNEL_CACHE[key]

    in_maps = []
    for e in range(E):
        n_e = len(idxs[e])
        xT_pad = np.zeros((D, cap), np.float32)
        xT_pad[:, :n_e] = x[idxs[e]].T
        # pretile to [P, n_chunks, KD, chunk]: row d = kt*P + p
        xt4 = np.ascontiguousarray(
            xT_pad.reshape(D // P, P, cap // chunk, chunk).transpose(1, 2, 0, 3))
        cw_pad = np.zeros(cap, np.float32)
        cw_pad[:n_e] = cws[e]
        in_maps.append({
            "x_t": xt4,
            "w1e": np.ascontiguousarray(w1[e]),
            "w2e": np.ascontiguousarray(w2[e]),
            "cw_t": np.ascontiguousarray(cw_pad.reshape(cap // P, P).T),
        })

    res = run_bass_kernel_spmd(nc, in_maps, core_ids=list(range(E)))
    globals()["LAST_RESULTS"] = res

    out = np.zeros((T, D), np.float32)
    for e in range(E):
        n_e = len(idxs[e])
        out[idxs[e]] += res.results[e]["y"][:n_e]

    return out.reshape(B, S, D), logits
